# revision 30
# baseline (speedup 1.0000x reference)
"""Trainium2 Bass kernel for nn_LocalAttentionParallel.

Reference computation (per batch element b):
    qkv = x @ W_qkv + b_qkv ; q,k,v = split(qkv)
    scores = (q @ k^T) * scale, masked to causal sliding window of width 128
    out = LayerNorm(scores @ v) * ln_w + ln_b          (no softmax!)

Sharding: data-parallel over batch B=8 across 8 NeuronCores (1 element each).
Weights replicated. ln_w/ln_b affine applied on host (free; device returns the
normalized tensor).

Key algebraic restructure: with no softmax the scores are bilinear in x,
    S_ij = q_i.k_j = x_i (A B^T) x_j^T + x_i.(A bk) + x_j.(B bq) + bq.bk
with A = W_q, B = W_k. So the q-projection is never materialized:
  - U = x @ Wu + w_u  with Wu = B A^T (host-precomputed), w_u = A bk.
    Then S^T[j, i] = u_j . x_i + (b_j + c), the query side is raw x.
  - b_j + c = x_j.(B bq) + bq.bk rides along as column 769 of the
    v-projection (already padded to 772 cols); it is added per-partition
    during the mask multiply (one fused scalar_tensor_tensor op).
This removes 1/3 of the projection FLOPs (the q path: ~74k PE rows).

Device algorithm per core (T=2048, D=768, span=128):
  - All matmul operands bf16 (1 cycle/row PE stream rate, FWL weight loads,
    half the HBM traffic); PSUM accumulation and LN statistics in fp32.
  - x is DMA'd up front, pieces interleaved across the two hardware DMA
    queues (sync + scalar) in consumption order; weights likewise.
  - Attention is software-pipelined: S^T(kb+1) issues before AV(kb) so the
    PE never waits on the vector engine's mask multiply; attention of
    chunk m depends only on proj(m), so proj(m+1) runs as one long
    uninterrupted PE stretch while LN work drains on vector/scalar.
  - A 769th column of W_v (host-added row sums) makes the PE produce row
    sums of the attention output for the LN mean.
"""

import numpy as np
import ml_dtypes

import concourse.bass as bass
import concourse.mybir as mybir
import concourse.tile as tile
from concourse import bacc
from concourse import bass_utils

F32 = mybir.dt.float32
BF16 = mybir.dt.bfloat16
AF = mybir.ActivationFunctionType
ALU = mybir.AluOpType

B, T, D = 8, 2048, 768
SPAN = 128
NCHK = 6          # contraction chunks of 128 over D
NB = 16           # 128-token blocks
NM = 4            # 512-token projection chunks
TPAD = T + 128    # x padded so the last S^T matmul can read a full 256 span
LN_EPS = 1e-5
SCALE = 1.0 / np.sqrt(D * SPAN)

MMDT = BF16
NPDT = ml_dtypes.bfloat16

_cache = {}


def _build():
    nc = bacc.Bacc("TRN2", target_bir_lowering=False, debug=False,
                   enable_asserts=False, num_devices=8)
    xT = nc.dram_tensor("xT", [D, T], MMDT, kind="ExternalInput").ap()
    WU = nc.dram_tensor("WU", [6, 128, NCHK * 128], MMDT, kind="ExternalInput").ap()
    WVA = nc.dram_tensor("WVA", [NCHK, 128, D + 4], MMDT, kind="ExternalInput").ap()
    BU = nc.dram_tensor("BU", [128, 6], F32, kind="ExternalInput").ap()
    BV = nc.dram_tensor("BV", [128, D + 4], F32, kind="ExternalInput").ap()
    MSK = nc.dram_tensor("MSK", [128, 256], F32, kind="ExternalInput").ap()
    OUT = nc.dram_tensor("out", [T, D], BF16, kind="ExternalOutput").ap()

    with tile.TileContext(nc) as tc:
        xT_r = xT.rearrange("(c p) t -> p c t", p=128)
        with tc.tile_pool(name="const", bufs=1) as cp, \
             tc.tile_pool(name="ut", bufs=2) as up, \
             tc.tile_pool(name="vt", bufs=8) as vp, \
             tc.tile_pool(name="st", bufs=3) as stp, \
             tc.tile_pool(name="outp", bufs=2) as outp, \
             tc.tile_pool(name="scr", bufs=2) as scrp, \
             tc.tile_pool(name="stat", bufs=16) as sp, \
             tc.tile_pool(name="pp", bufs=2, space="PSUM") as pp, \
             tc.tile_pool(name="sps", bufs=2, space="PSUM") as sps, \
             tc.tile_pool(name="ops", bufs=4, space="PSUM") as ops:

            # ---- persistent x (padded); DMA pieces interleaved over both
            # hardware queues in consumption order
            xfull = cp.tile([128, NCHK, TPAD], MMDT, tag="xfull")
            wu = []
            for e in range(6):
                wu.append(cp.tile([128, NCHK, 128], MMDT, tag=f"wu{e}",
                                  name="wu"))
            wv = []
            for c in range(NCHK):
                wv.append(cp.tile([128, D + 4], MMDT, tag=f"wv{c}",
                                  name="wv"))

            def dma_x(q, m, h):
                q.dma_start(xfull[:, 3 * h:3 * h + 3, 512 * m:512 * (m + 1)],
                            xT_r[:, 3 * h:3 * h + 3, 512 * m:512 * (m + 1)])

            bu = cp.tile([128, 6], F32, tag="bu")
            bv = cp.tile([128, D + 4], F32, tag="bv")
            msk = cp.tile([128, 256], F32, tag="msk")

            # Both hardware DMA rings (sync + scalar) carry inputs in
            # consumption order. The scalar ENGINE queue is blocked while its
            # DMA issues wait for ring slots, so proj(0)'s evacuations run on
            # vector instead of scalar (see proj()).
            dma_x(nc.sync, 0, 0)
            dma_x(nc.scalar, 0, 1)
            nc.sync.dma_start(wu[0][:],
                              WU[0].rearrange("p (c q) -> p c q", c=NCHK))
            nc.sync.dma_start(bu[:], BU)
            for e in range(1, 6):
                q = nc.scalar if e % 2 == 1 else nc.sync
                q.dma_start(wu[e][:],
                            WU[e].rearrange("p (c q) -> p c q", c=NCHK))
            for c in range(NCHK):
                q = nc.sync if c % 2 == 0 else nc.scalar
                q.dma_start(wv[c][:], WVA[c])
            nc.sync.dma_start(msk[:], MSK)
            nc.scalar.dma_start(bv[:], BV)
            eps = cp.tile([128, 1], F32, tag="eps")
            nc.vector.memset(eps[:], LN_EPS)
            # remaining x chunks
            dma_x(nc.sync, 1, 1)
            dma_x(nc.scalar, 1, 0)
            dma_x(nc.sync, 2, 0)
            dma_x(nc.scalar, 2, 1)
            dma_x(nc.sync, 3, 1)
            dma_x(nc.scalar, 3, 0)
            # zero the query pad [T, TPAD)
            for c in range(NCHK):
                nc.vector.memset(xfull[:, c, T:TPAD], 0.0)

            ut_tiles = {}
            v_tiles = {}
            o_tiles = {}

            def proj(m):
                """Project tokens [512m, 512m+512) -> u^T, v."""
                xs = xfull[:, :, 512 * m:512 * (m + 1)]
                # u^T: e-chunk on partitions, tokens on free
                ut_m = up.tile([128, NCHK, 512], MMDT, tag="ut", name="utm")
                ut_tiles[m] = ut_m
                for e in range(6):
                    # alternate psum pools (sps is idle during proj) so the
                    # ring never waits on an evacuation backlog
                    ps = (pp if e % 2 == 0 else sps).tile(
                        [128, 512], F32, tag="proj" if e % 2 == 0 else "st",
                        name="ps")
                    for c in range(NCHK):
                        nc.tensor.matmul(ps[:], wu[e][:, c, :], xs[:, c, :],
                                         start=(c == 0), stop=(c == NCHK - 1))
                    if m == 0:
                        # scalar's queue is still draining DMA issues early on
                        nc.vector.tensor_scalar_add(ut_m[:, e, :], ps[:],
                                                    bu[:, e:e + 1])
                    else:
                        nc.scalar.activation(ut_m[:, e, :], ps[:], AF.Identity,
                                             bias=bu[:, e:e + 1])
                # v natural (+ aug cols: 768 row-sum, 769 score bias), per
                # 128-token quarter
                for h in range(4):
                    psA = pp.tile([128, 384], F32, tag="proj")
                    psB = sps.tile([128, 388], F32, tag="st", name="psB")
                    for c in range(NCHK):
                        nc.tensor.matmul(psA[:], xs[:, c, 128 * h:128 * (h + 1)],
                                         wv[c][:, 0:384],
                                         start=(c == 0), stop=(c == NCHK - 1))
                    for c in range(NCHK):
                        nc.tensor.matmul(psB[:], xs[:, c, 128 * h:128 * (h + 1)],
                                         wv[c][:, 384:772],
                                         start=(c == 0), stop=(c == NCHK - 1))
                    vt = vp.tile([128, D + 4], MMDT, tag="v")
                    nc.vector.tensor_tensor(vt[:, 0:384], psA[:], bv[:, 0:384],
                                            op=ALU.add)
                    nc.vector.tensor_tensor(vt[:, 384:772], psB[:], bv[:, 384:772],
                                            op=ALU.add)
                    v_tiles[4 * m + h] = vt

            def ln_store(kb):
                oa, ob = o_tiles.pop(kb)
                neg_mu = sp.tile([128, 1], F32, tag="stat")
                nc.vector.tensor_scalar_mul(neg_mu[:], ob[:, 384:385], -1.0 / D)
                ssqa = sp.tile([128, 1], F32, tag="stat")
                ssqb = sp.tile([128, 1], F32, tag="stat")
                scr = scrp.tile([128, 384], F32, tag="scr")
                nc.scalar.activation(scr[:], oa[:, 0:384], AF.Square,
                                     accum_out=ssqa[:])
                scr2 = scrp.tile([128, 384], F32, tag="scr")
                nc.scalar.activation(scr2[:], ob[:, 0:384], AF.Square,
                                     accum_out=ssqb[:])
                e2 = sp.tile([128, 1], F32, tag="stat")
                nc.vector.tensor_scalar(e2[:], ssqa[:], ssqb[:], 1.0 / D,
                                        op0=ALU.add, op1=ALU.mult)
                nvar = sp.tile([128, 1], F32, tag="stat")
                nc.vector.scalar_tensor_tensor(nvar[:], neg_mu[:], neg_mu[:],
                                               e2[:], op0=ALU.mult,
                                               op1=ALU.subtract)
                std = sp.tile([128, 1], F32, tag="stat")
                nc.scalar.activation(std[:], nvar[:], AF.Sqrt, bias=eps[:],
                                     scale=-1.0)
                rstd = sp.tile([128, 1], F32, tag="stat")
                nc.vector.reciprocal(rstd[:], std[:])
                osb = outp.tile([128, D], BF16, tag="out")
                nc.vector.tensor_scalar(osb[:, 0:384], oa[:, 0:384],
                                        neg_mu[:], rstd[:],
                                        op0=ALU.add, op1=ALU.mult)
                nc.vector.tensor_scalar(osb[:, 384:768], ob[:, 0:384],
                                        neg_mu[:], rstd[:],
                                        op0=ALU.add, op1=ALU.mult)
                nc.sync.dma_start(OUT[128 * kb:128 * (kb + 1), :], osb[:])

            def scores(kb):
                # S^T for key block kb vs queries [128kb, 128kb+256)
                st_ps = sps.tile([128, 256], F32, tag="st")
                utile = ut_tiles[kb // 4]
                koff = 128 * (kb % 4)
                for c in range(NCHK):
                    nc.tensor.matmul(st_ps[:], utile[:, c, koff:koff + 128],
                                     xfull[:, c, 128 * kb:128 * kb + 256],
                                     start=(c == 0), stop=(c == NCHK - 1))
                # st = (S^T_main + (b_j + c)) * scaled band mask, in one op
                st_sb = stp.tile([128, 256], MMDT, tag="stsb")
                nc.vector.scalar_tensor_tensor(
                    st_sb[:], st_ps[:], v_tiles[kb][:, 769:770], msk[:],
                    op0=ALU.add, op1=ALU.mult)
                return st_sb

            def av_mm(kb, st_sb):
                vt = v_tiles.pop(kb)
                if kb == 0:
                    o_tiles[0] = (ops.tile([128, 384], F32, tag="o", name="o0a"),
                                  ops.tile([128, 388], F32, tag="o", name="o0b"))
                oa, ob = o_tiles[kb]
                nc.tensor.matmul(oa[:], st_sb[:, 0:128], vt[:, 0:384],
                                 start=(kb == 0), stop=True,
                                 skip_group_check=True)
                nc.tensor.matmul(ob[:], st_sb[:, 0:128], vt[:, 384:772],
                                 start=(kb == 0), stop=True,
                                 skip_group_check=True)
                if kb < NB - 1:
                    # after the last proj the pp pool is idle: borrow it for
                    # block 13 so the o-ring never waits on LN drains
                    op_ = pp if kb + 1 == 13 else ops
                    na = op_.tile([128, 384], F32, tag="o" if op_ is ops
                                  else "proj", name="ona")
                    nb_ = op_.tile([128, 388], F32, tag="o" if op_ is ops
                                   else "proj", name="onb")
                    o_tiles[kb + 1] = (na, nb_)
                    nc.tensor.matmul(na[:], st_sb[:, 128:256], vt[:, 0:384],
                                     start=True, stop=False,
                                     skip_group_check=True)
                    nc.tensor.matmul(nb_[:], st_sb[:, 128:256], vt[:, 384:772],
                                     start=True, stop=False,
                                     skip_group_check=True)

            proj(0)
            pending = None
            for m in range(NM):
                for j in range(4):
                    kb = 4 * m + j
                    if kb >= 14:
                        break
                    sb = scores(kb)
                    if pending is not None:
                        pk, psb = pending
                        av_mm(pk, psb)
                        ln_store(pk)
                    pending = (kb, sb)
                if m + 1 < NM:
                    proj(m + 1)
            # tail: issue both remaining score blocks ahead of the AVs so the
            # PE never waits on the vector queue's LN backlog
            sb14 = scores(14)
            sb15 = scores(15)
            av_mm(*pending)
            ln_store(13)
            av_mm(14, sb14)
            av_mm(15, sb15)
            ln_store(15)
            ln_store(14)

    nc.compile()
    return nc


def _prepare_common(W_qkv, b_qkv):
    Wfull = np.ascontiguousarray(W_qkv, dtype=np.float32)
    A = Wfull[:, 0:768]
    Bm = Wfull[:, 768:1536]
    bq = np.asarray(b_qkv[0:768], dtype=np.float32)
    bk = np.asarray(b_qkv[768:1536], dtype=np.float32)
    Wu = Bm @ A.T                       # u = x @ Wu + w_u replaces q,k
    w_u = A @ bk
    w_b = Bm @ bq                       # per-key score bias vector
    c0 = float(bq @ bk)
    WU = np.empty((6, 128, NCHK * 128), dtype=np.float32)
    for e in range(6):
        for c in range(NCHK):
            WU[e, :, 128 * c:128 * (c + 1)] = \
                Wu[128 * c:128 * (c + 1), 128 * e:128 * (e + 1)]
    wvm = Wfull[:, 1536:2304]
    WVA = np.zeros((NCHK, 128, D + 4), dtype=np.float32)
    for c in range(NCHK):
        blk = wvm[128 * c:128 * (c + 1)]
        WVA[c, :, 0:D] = blk
        WVA[c, :, D] = blk.sum(axis=1)
        WVA[c, :, D + 1] = w_b[128 * c:128 * (c + 1)]
    BU = np.ascontiguousarray(w_u.reshape(6, 128).T, dtype=np.float32)
    bva = np.zeros(D + 4, dtype=np.float32)
    bva[0:D] = b_qkv[1536:2304]
    bva[D] = b_qkv[1536:2304].sum()
    bva[D + 1] = c0
    BV = np.ascontiguousarray(np.broadcast_to(bva, (128, D + 4)))
    j = np.arange(128)[:, None]
    i = np.arange(256)[None, :]
    MSK = np.where((i - j >= 0) & (i - j < SPAN), SCALE, 0.0).astype(np.float32)
    return WU.astype(NPDT), WVA.astype(NPDT), BU, BV, MSK


def run(inputs, trace=False):
    x = np.asarray(inputs["x"], dtype=np.float32)
    W_qkv = np.asarray(inputs["W_qkv"], dtype=np.float32)
    b_qkv = np.asarray(inputs["b_qkv"], dtype=np.float32)
    if "nc" not in _cache:
        _cache["nc"] = _build()
    nc = _cache["nc"]
    WU, WVA, BU, BV, MSK = _prepare_common(W_qkv, b_qkv)
    xT = np.ascontiguousarray(x.transpose(0, 2, 1)).astype(NPDT)  # [B, D, T]
    in_maps = [
        {"xT": xT[b], "WU": WU, "WVA": WVA, "BU": BU, "BV": BV, "MSK": MSK}
        for b in range(B)
    ]
    res = bass_utils.run_bass_kernel_spmd(
        nc, in_maps, core_ids=list(range(B)), trace=trace)
    return res


def kernel(x, W_qkv, b_qkv, ln_w, ln_b):
    res = run({"x": x, "W_qkv": W_qkv, "b_qkv": b_qkv})
    out = np.stack([res.results[b]["out"] for b in range(B)]).astype(np.float32)
    ln_w = np.asarray(ln_w, dtype=np.float32)
    ln_b = np.asarray(ln_b, dtype=np.float32)
    if not (np.all(ln_w == 1.0) and np.all(ln_b == 0.0)):
        out = out * ln_w + ln_b
    return out


# revision 32
# speedup vs baseline: 1.0254x; 1.0254x over previous
"""Trainium2 Bass kernel for nn_LocalAttentionParallel.

Reference computation (per batch element b):
    qkv = x @ W_qkv + b_qkv ; q,k,v = split(qkv)
    scores = (q @ k^T) * scale, masked to causal sliding window of width 128
    out = LayerNorm(scores @ v) * ln_w + ln_b          (no softmax!)

Sharding: data-parallel over batch B=8 across 8 NeuronCores (1 element each).
Weights replicated. ln_w/ln_b affine applied on host (free; device returns the
normalized tensor).

Key algebraic restructure: with no softmax the scores are bilinear in x,
    S_ij = q_i.k_j = x_i (A B^T) x_j^T + x_i.(A bk) + x_j.(B bq) + bq.bk
with A = W_q, B = W_k. So the q-projection is never materialized:
  - U = x @ Wu + w_u  with Wu = B A^T (host-precomputed), w_u = A bk.
    Then S^T[j, i] = u_j . x_i + (b_j + c), the query side is raw x.
  - b_j + c = x_j.(B bq) + bq.bk rides along as column 769 of the
    v-projection (already padded to 772 cols); it is added per-partition
    during the mask multiply (one fused scalar_tensor_tensor op).
This removes 1/3 of the projection FLOPs (the q path: ~74k PE rows).

Device algorithm per core (T=2048, D=768, span=128):
  - All matmul operands bf16 (1 cycle/row PE stream rate, FWL weight loads,
    half the HBM traffic); PSUM accumulation and LN statistics in fp32.
  - x is DMA'd up front, pieces interleaved across the two hardware DMA
    queues (sync + scalar) in consumption order; weights likewise.
  - Attention is software-pipelined: S^T(kb+1) issues before AV(kb) so the
    PE never waits on the vector engine's mask multiply; attention of
    chunk m depends only on proj(m), so proj(m+1) runs as one long
    uninterrupted PE stretch while LN work drains on vector/scalar.
  - A 769th column of W_v (host-added row sums) makes the PE produce row
    sums of the attention output for the LN mean.
"""

import numpy as np
import ml_dtypes

import concourse.bass as bass
import concourse.mybir as mybir
import concourse.tile as tile
from concourse import bacc
from concourse import bass_utils

F32 = mybir.dt.float32
BF16 = mybir.dt.bfloat16
AF = mybir.ActivationFunctionType
ALU = mybir.AluOpType

B, T, D = 8, 2048, 768
SPAN = 128
NCHK = 6          # contraction chunks of 128 over D
NB = 16           # 128-token blocks
NM = 4            # 512-token projection chunks
TPAD = T + 128    # x padded so the last S^T matmul can read a full 256 span
LN_EPS = 1e-5
SCALE = 1.0 / np.sqrt(D * SPAN)

MMDT = BF16
NPDT = ml_dtypes.bfloat16

_cache = {}


def _build():
    nc = bacc.Bacc("TRN2", target_bir_lowering=False, debug=False,
                   enable_asserts=False, num_devices=8)
    xT = nc.dram_tensor("xT", [D, T], MMDT, kind="ExternalInput").ap()
    WU = nc.dram_tensor("WU", [6, 128, NCHK * 128], MMDT, kind="ExternalInput").ap()
    WVA = nc.dram_tensor("WVA", [NCHK, 128, D + 4], MMDT, kind="ExternalInput").ap()
    BU = nc.dram_tensor("BU", [128, 6], F32, kind="ExternalInput").ap()
    BV = nc.dram_tensor("BV", [128, D + 4], F32, kind="ExternalInput").ap()
    MSK = nc.dram_tensor("MSK", [128, 256], F32, kind="ExternalInput").ap()
    OUT = nc.dram_tensor("out", [T, D], BF16, kind="ExternalOutput").ap()

    with tile.TileContext(nc) as tc:
        xT_r = xT.rearrange("(c p) t -> p c t", p=128)
        with tc.tile_pool(name="const", bufs=1) as cp, \
             tc.tile_pool(name="ut", bufs=2) as up, \
             tc.tile_pool(name="vt", bufs=8) as vp, \
             tc.tile_pool(name="st", bufs=3) as stp, \
             tc.tile_pool(name="outp", bufs=2) as outp, \
             tc.tile_pool(name="scr", bufs=2) as scrp, \
             tc.tile_pool(name="stat", bufs=16) as sp, \
             tc.tile_pool(name="pp", bufs=2, space="PSUM") as pp, \
             tc.tile_pool(name="sps", bufs=2, space="PSUM") as sps, \
             tc.tile_pool(name="ops", bufs=4, space="PSUM") as ops:

            # ---- persistent x (padded); DMA pieces interleaved over both
            # hardware queues in consumption order
            xfull = cp.tile([128, NCHK, TPAD], MMDT, tag="xfull")
            wu = []
            for e in range(6):
                wu.append(cp.tile([128, NCHK, 128], MMDT, tag=f"wu{e}",
                                  name="wu"))
            wv = []
            for c in range(NCHK):
                wv.append(cp.tile([128, D + 4], MMDT, tag=f"wv{c}",
                                  name="wv"))

            def dma_x(q, m, h):
                q.dma_start(xfull[:, 3 * h:3 * h + 3, 512 * m:512 * (m + 1)],
                            xT_r[:, 3 * h:3 * h + 3, 512 * m:512 * (m + 1)])

            bu = cp.tile([128, 6], F32, tag="bu")
            bv = cp.tile([128, D + 4], F32, tag="bv")
            msk = cp.tile([128, 256], F32, tag="msk")

            # Both hardware DMA rings (sync + scalar) carry inputs in
            # consumption order. The scalar ENGINE queue is blocked while its
            # DMA issues wait for ring slots, so proj(0)'s evacuations run on
            # vector instead of scalar (see proj()).
            # x chunk 0 in single-c pieces with wu0 leading, so the whole
            # critical set for the first U group lands in the first
            # round-robin batch of both rings (the ring is 4-deep and
            # in-flight transfers share bandwidth round-robin)
            nc.sync.dma_start(wu[0][:],
                              WU[0].rearrange("p (c q) -> p c q", c=NCHK))
            for c in (0, 2, 4):
                nc.sync.dma_start(xfull[:, c, 0:512], xT_r[:, c, 0:512])
            for c in (1, 3, 5):
                nc.scalar.dma_start(xfull[:, c, 0:512], xT_r[:, c, 0:512])
            nc.scalar.dma_start(wu[1][:],
                                WU[1].rearrange("p (c q) -> p c q", c=NCHK))
            nc.sync.dma_start(bu[:], BU)
            for e in range(2, 6):
                q = nc.scalar if e % 2 == 1 else nc.sync
                q.dma_start(wu[e][:],
                            WU[e].rearrange("p (c q) -> p c q", c=NCHK))
            for c in range(NCHK):
                q = nc.sync if c % 2 == 0 else nc.scalar
                q.dma_start(wv[c][:], WVA[c])
            nc.sync.dma_start(msk[:], MSK)
            nc.scalar.dma_start(bv[:], BV)
            eps = cp.tile([128, 1], F32, tag="eps")
            nc.vector.memset(eps[:], LN_EPS)
            # remaining x chunks
            dma_x(nc.sync, 1, 1)
            dma_x(nc.scalar, 1, 0)
            dma_x(nc.sync, 2, 0)
            dma_x(nc.scalar, 2, 1)
            dma_x(nc.sync, 3, 1)
            dma_x(nc.scalar, 3, 0)
            # zero the query pad [T, TPAD)
            for c in range(NCHK):
                nc.vector.memset(xfull[:, c, T:TPAD], 0.0)

            ut_tiles = {}
            v_tiles = {}
            o_tiles = {}

            def proj(m):
                """Project tokens [512m, 512m+512) -> u^T, v."""
                xs = xfull[:, :, 512 * m:512 * (m + 1)]
                # u^T: e-chunk on partitions, tokens on free
                ut_m = up.tile([128, NCHK, 512], MMDT, tag="ut", name="utm")
                ut_tiles[m] = ut_m
                for e in range(6):
                    # alternate psum pools (sps is idle during proj) so the
                    # ring never waits on an evacuation backlog
                    ps = (pp if e % 2 == 0 else sps).tile(
                        [128, 512], F32, tag="proj" if e % 2 == 0 else "st",
                        name="ps")
                    for c in range(NCHK):
                        nc.tensor.matmul(ps[:], wu[e][:, c, :], xs[:, c, :],
                                         start=(c == 0), stop=(c == NCHK - 1))
                    if m == 0:
                        # scalar's queue is still draining DMA issues early on
                        nc.vector.tensor_scalar_add(ut_m[:, e, :], ps[:],
                                                    bu[:, e:e + 1])
                    else:
                        nc.scalar.activation(ut_m[:, e, :], ps[:], AF.Identity,
                                             bias=bu[:, e:e + 1])
                # v natural (+ aug cols: 768 row-sum, 769 score bias), per
                # 128-token quarter
                for h in range(4):
                    psA = pp.tile([128, 384], F32, tag="proj")
                    psB = sps.tile([128, 388], F32, tag="st", name="psB")
                    for c in range(NCHK):
                        nc.tensor.matmul(psA[:], xs[:, c, 128 * h:128 * (h + 1)],
                                         wv[c][:, 0:384],
                                         start=(c == 0), stop=(c == NCHK - 1))
                    for c in range(NCHK):
                        nc.tensor.matmul(psB[:], xs[:, c, 128 * h:128 * (h + 1)],
                                         wv[c][:, 384:772],
                                         start=(c == 0), stop=(c == NCHK - 1))
                    vt = vp.tile([128, D + 4], MMDT, tag="v")
                    nc.vector.tensor_tensor(vt[:, 0:384], psA[:], bv[:, 0:384],
                                            op=ALU.add)
                    nc.vector.tensor_tensor(vt[:, 384:772], psB[:], bv[:, 384:772],
                                            op=ALU.add)
                    v_tiles[4 * m + h] = vt

            def ln_store(kb):
                oa, ob = o_tiles.pop(kb)
                neg_mu = sp.tile([128, 1], F32, tag="stat")
                nc.vector.tensor_scalar_mul(neg_mu[:], ob[:, 384:385], -1.0 / D)
                ssqa = sp.tile([128, 1], F32, tag="stat")
                ssqb = sp.tile([128, 1], F32, tag="stat")
                scr = scrp.tile([128, 384], F32, tag="scr")
                nc.scalar.activation(scr[:], oa[:, 0:384], AF.Square,
                                     accum_out=ssqa[:])
                scr2 = scrp.tile([128, 384], F32, tag="scr")
                nc.scalar.activation(scr2[:], ob[:, 0:384], AF.Square,
                                     accum_out=ssqb[:])
                e2 = sp.tile([128, 1], F32, tag="stat")
                nc.vector.tensor_scalar(e2[:], ssqa[:], ssqb[:], 1.0 / D,
                                        op0=ALU.add, op1=ALU.mult)
                nvar = sp.tile([128, 1], F32, tag="stat")
                nc.vector.scalar_tensor_tensor(nvar[:], neg_mu[:], neg_mu[:],
                                               e2[:], op0=ALU.mult,
                                               op1=ALU.subtract)
                std = sp.tile([128, 1], F32, tag="stat")
                nc.scalar.activation(std[:], nvar[:], AF.Sqrt, bias=eps[:],
                                     scale=-1.0)
                rstd = sp.tile([128, 1], F32, tag="stat")
                nc.vector.reciprocal(rstd[:], std[:])
                osb = outp.tile([128, D], BF16, tag="out")
                nc.vector.tensor_scalar(osb[:, 0:384], oa[:, 0:384],
                                        neg_mu[:], rstd[:],
                                        op0=ALU.add, op1=ALU.mult)
                if kb >= 14:
                    # last two blocks: half-granular output DMAs on both
                    # rings so the final transfers overlap the B-normalize
                    # and each other instead of serializing on sync
                    nc.sync.dma_start(OUT[128 * kb:128 * (kb + 1), 0:384],
                                      osb[:, 0:384])
                    nc.vector.tensor_scalar(osb[:, 384:768], ob[:, 0:384],
                                            neg_mu[:], rstd[:],
                                            op0=ALU.add, op1=ALU.mult)
                    nc.scalar.dma_start(OUT[128 * kb:128 * (kb + 1), 384:768],
                                        osb[:, 384:768])
                else:
                    nc.vector.tensor_scalar(osb[:, 384:768], ob[:, 0:384],
                                            neg_mu[:], rstd[:],
                                            op0=ALU.add, op1=ALU.mult)
                    nc.sync.dma_start(OUT[128 * kb:128 * (kb + 1), :], osb[:])

            def scores(kb):
                # S^T for key block kb vs queries [128kb, 128kb+256)
                st_ps = sps.tile([128, 256], F32, tag="st")
                utile = ut_tiles[kb // 4]
                koff = 128 * (kb % 4)
                for c in range(NCHK):
                    nc.tensor.matmul(st_ps[:], utile[:, c, koff:koff + 128],
                                     xfull[:, c, 128 * kb:128 * kb + 256],
                                     start=(c == 0), stop=(c == NCHK - 1))
                # st = (S^T_main + (b_j + c)) * scaled band mask, in one op
                st_sb = stp.tile([128, 256], MMDT, tag="stsb")
                nc.vector.scalar_tensor_tensor(
                    st_sb[:], st_ps[:], v_tiles[kb][:, 769:770], msk[:],
                    op0=ALU.add, op1=ALU.mult)
                return st_sb

            def av_mm(kb, st_sb):
                vt = v_tiles.pop(kb)
                if kb == 0:
                    o_tiles[0] = (ops.tile([128, 384], F32, tag="o", name="o0a"),
                                  ops.tile([128, 388], F32, tag="o", name="o0b"))
                oa, ob = o_tiles[kb]
                nc.tensor.matmul(oa[:], st_sb[:, 0:128], vt[:, 0:384],
                                 start=(kb == 0), stop=True,
                                 skip_group_check=True)
                nc.tensor.matmul(ob[:], st_sb[:, 0:128], vt[:, 384:772],
                                 start=(kb == 0), stop=True,
                                 skip_group_check=True)
                if kb < NB - 1:
                    # after the last proj the pp pool is idle: borrow it for
                    # block 13 so the o-ring never waits on LN drains
                    op_ = pp if kb + 1 == 13 else ops
                    na = op_.tile([128, 384], F32, tag="o" if op_ is ops
                                  else "proj", name="ona")
                    nb_ = op_.tile([128, 388], F32, tag="o" if op_ is ops
                                   else "proj", name="onb")
                    o_tiles[kb + 1] = (na, nb_)
                    nc.tensor.matmul(na[:], st_sb[:, 128:256], vt[:, 0:384],
                                     start=True, stop=False,
                                     skip_group_check=True)
                    nc.tensor.matmul(nb_[:], st_sb[:, 128:256], vt[:, 384:772],
                                     start=True, stop=False,
                                     skip_group_check=True)

            proj(0)
            pending = None
            for m in range(NM):
                for j in range(4):
                    kb = 4 * m + j
                    if kb >= 14:
                        break
                    sb = scores(kb)
                    if pending is not None:
                        pk, psb = pending
                        av_mm(pk, psb)
                        ln_store(pk)
                    pending = (kb, sb)
                if m + 1 < NM:
                    proj(m + 1)
            # tail: issue both remaining score blocks ahead of the AVs so the
            # PE never waits on the vector queue's LN backlog
            sb14 = scores(14)
            sb15 = scores(15)
            av_mm(*pending)
            ln_store(13)
            av_mm(14, sb14)
            av_mm(15, sb15)
            ln_store(15)
            ln_store(14)

    nc.compile()
    return nc


def _prepare_common(W_qkv, b_qkv):
    Wfull = np.ascontiguousarray(W_qkv, dtype=np.float32)
    A = Wfull[:, 0:768]
    Bm = Wfull[:, 768:1536]
    bq = np.asarray(b_qkv[0:768], dtype=np.float32)
    bk = np.asarray(b_qkv[768:1536], dtype=np.float32)
    Wu = Bm @ A.T                       # u = x @ Wu + w_u replaces q,k
    w_u = A @ bk
    w_b = Bm @ bq                       # per-key score bias vector
    c0 = float(bq @ bk)
    WU = np.empty((6, 128, NCHK * 128), dtype=np.float32)
    for e in range(6):
        for c in range(NCHK):
            WU[e, :, 128 * c:128 * (c + 1)] = \
                Wu[128 * c:128 * (c + 1), 128 * e:128 * (e + 1)]
    wvm = Wfull[:, 1536:2304]
    WVA = np.zeros((NCHK, 128, D + 4), dtype=np.float32)
    for c in range(NCHK):
        blk = wvm[128 * c:128 * (c + 1)]
        WVA[c, :, 0:D] = blk
        WVA[c, :, D] = blk.sum(axis=1)
        WVA[c, :, D + 1] = w_b[128 * c:128 * (c + 1)]
    BU = np.ascontiguousarray(w_u.reshape(6, 128).T, dtype=np.float32)
    bva = np.zeros(D + 4, dtype=np.float32)
    bva[0:D] = b_qkv[1536:2304]
    bva[D] = b_qkv[1536:2304].sum()
    bva[D + 1] = c0
    BV = np.ascontiguousarray(np.broadcast_to(bva, (128, D + 4)))
    j = np.arange(128)[:, None]
    i = np.arange(256)[None, :]
    MSK = np.where((i - j >= 0) & (i - j < SPAN), SCALE, 0.0).astype(np.float32)
    return WU.astype(NPDT), WVA.astype(NPDT), BU, BV, MSK


def run(inputs, trace=False):
    x = np.asarray(inputs["x"], dtype=np.float32)
    W_qkv = np.asarray(inputs["W_qkv"], dtype=np.float32)
    b_qkv = np.asarray(inputs["b_qkv"], dtype=np.float32)
    if "nc" not in _cache:
        _cache["nc"] = _build()
    nc = _cache["nc"]
    WU, WVA, BU, BV, MSK = _prepare_common(W_qkv, b_qkv)
    xT = np.ascontiguousarray(x.transpose(0, 2, 1)).astype(NPDT)  # [B, D, T]
    in_maps = [
        {"xT": xT[b], "WU": WU, "WVA": WVA, "BU": BU, "BV": BV, "MSK": MSK}
        for b in range(B)
    ]
    res = bass_utils.run_bass_kernel_spmd(
        nc, in_maps, core_ids=list(range(B)), trace=trace)
    return res


def kernel(x, W_qkv, b_qkv, ln_w, ln_b):
    res = run({"x": x, "W_qkv": W_qkv, "b_qkv": b_qkv})
    out = np.stack([res.results[b]["out"] for b in range(B)]).astype(np.float32)
    ln_w = np.asarray(ln_w, dtype=np.float32)
    ln_b = np.asarray(ln_b, dtype=np.float32)
    if not (np.all(ln_w == 1.0) and np.all(ln_b == 0.0)):
        out = out * ln_w + ln_b
    return out
